# revision 13
# baseline (speedup 1.0000x reference)
"""GCN layer on 8 Trainium2 NeuronCores (Bass/Tile).

out = relu( (D^-1/2 A D^-1/2 x) W^T + b ),  A = scatter of 1.6M directed edges.

Strategy (sharding hint: partition edges across devices, replicated node
features, local segment-sums):
  - Host does graph-STRUCTURE prep only: degree/normalization scalars
    (pure function of edge_index), edge bucketing/padding, index layouts.
  - Each core owns a contiguous 1/8 dst-node range and ALL edges targeting
    it, so per-node aggregates complete locally - no collective needed.
  - Device does all feature work: scale x by deg^-1/2 into a bf16
    duplicated table xs (256B rows), per-edge dma_gather of source rows
    (4 SWDGE queues), scatter-add via one-hot matmul accumulation in PSUM
    per 128-dst bin, fused dst-scale + 64x64 linear + bias + relu epilogue.
  - The instruction stream is identical across cores (uniform cell plan =
    max tile count over the 8 cores per (bin, chunk) cell); per-core data
    (indices, one-hot targets) are runtime inputs, so one SPMD NEFF runs
    on all 8 cores via shard_map.
"""
import numpy as np
import ml_dtypes
from contextlib import ExitStack

N_NODES = 100000
N_EDGES = 1600000
D = 64
N_CORES = 8

NPAD = 100352            # 784 * 128 node rows (zero padded)
NCHUNKS = 4
CHUNK_N = 25088          # nodes per gather source chunk (int16-indexable)
NXT = NPAD // 128        # 784 node tiles
CORE_N = N_NODES // N_CORES   # 12500 dst nodes per core
BIN = 128
NBINS = (CORE_N + BIN - 1) // BIN   # 98 (last bin has 84 rows)
GROUP = 6                # bins per PSUM group (6 agg banks + 2 out banks)
XS_STEP = 28             # node tiles per xs-phase streaming step (196 = 7*28)


def _host_prep(x, edge_index, W, b):
    row = np.asarray(edge_index[0], dtype=np.int64)
    col = np.asarray(edge_index[1], dtype=np.int64)

    # --- structure-only prep (degree -> normalization scalars) ---
    deg = np.bincount(row, minlength=N_NODES).astype(np.float64)
    with np.errstate(divide="ignore"):
        dis = np.where(deg > 0, deg ** -0.5, 0.0).astype(np.float32)  # [N]

    dis_pad = np.zeros(NPAD, dtype=np.float32)
    dis_pad[:N_NODES] = dis
    dis_pt = np.ascontiguousarray(dis_pad.reshape(NXT, 128).T)  # [128, NXT]

    x_pad = np.zeros((NPAD, D), dtype=np.float32)
    x_pad[:N_NODES] = np.asarray(x, dtype=np.float32)

    W_ext = np.zeros((65, 64), dtype=np.float32)
    W_ext[:64] = np.asarray(W, dtype=np.float32).T     # [in f, out f]
    W_ext[64] = np.asarray(b, dtype=np.float32)
    W_ext = W_ext.astype(ml_dtypes.bfloat16)

    # --- edge bucketing: core (dst range), bin (dst/128), chunk (src/25088)
    core = row // CORE_N
    binid = (row % CORE_N) // BIN
    chunk = col // CHUNK_N
    cnt = np.zeros((N_CORES, NBINS, NCHUNKS), dtype=np.int64)
    np.add.at(cnt, (core, binid, chunk), 1)

    T_bc = np.maximum(1, -(-cnt.max(axis=0) // 128))   # [NBINS, NCHUNKS] tiles

    # plan order: for each group of GROUP bins, for each chunk, bins in order
    groups = [list(range(g, min(g + GROUP, NBINS))) for g in range(0, NBINS, GROUP)]
    cell_rank = np.zeros((NBINS, NCHUNKS), dtype=np.int64)
    cell_off = np.zeros((NBINS, NCHUNKS), dtype=np.int64)  # tile offset
    calls = []   # (chunk, idx_tile_off, [(bin, ntiles), ...])
    t_off = 0
    r = 0
    for g in groups:
        for k in range(NCHUNKS):
            cells = []
            call_off = t_off
            for bb in g:
                cell_rank[bb, k] = r
                cell_off[bb, k] = t_off
                r += 1
                cells.append((bb, int(T_bc[bb, k])))
                t_off += int(T_bc[bb, k])
            calls.append((k, call_off, cells))
    NT = t_off
    E_plan = NT * 128

    # --- per-core index/dstloc packing into the uniform plan ---
    rank_of = cell_rank[binid, chunk]           # [E] cell rank per edge
    order = np.argsort(core * (NBINS * NCHUNKS * 2) + rank_of, kind="stable")
    r_s, c_s = row[order], col[order]
    core_s, rank_s = core[order], rank_of[order]
    bin_s, chunk_s = binid[order], chunk[order]
    # within-(core,cell) running index
    key = core_s * (NBINS * NCHUNKS) + rank_s
    first = np.r_[True, key[1:] != key[:-1]]
    grp_start = np.flatnonzero(first)
    within = np.arange(len(key)) - np.repeat(grp_start, np.diff(np.r_[grp_start, len(key)]))

    cell_off_flat = cell_off[bin_s, chunk_s]
    pos = cell_off_flat * 128 + within          # slot within the core's plan

    eidx = np.zeros((N_CORES, E_plan), dtype=np.int16)
    dstl = np.full((N_CORES, E_plan), -1.0, dtype=np.float32)
    eidx[core_s, pos] = (c_s - chunk_s * CHUNK_N).astype(np.int16)
    dstl[core_s, pos] = (r_s - core_s * CORE_N - bin_s * BIN).astype(np.float32)

    # wrap idxs: idx i -> [i % 16, i // 16], replicated to 128 partitions
    eidx_w = np.ascontiguousarray(
        np.tile(eidx.reshape(N_CORES, E_plan // 16, 16).transpose(0, 2, 1), (1, 8, 1))
    )
    dstl_pt = np.ascontiguousarray(
        dstl.reshape(N_CORES, NT, 128).transpose(0, 2, 1)
    ).astype(ml_dtypes.bfloat16)

    dis_dst = np.zeros((N_CORES, NBINS * BIN), dtype=np.float32)
    for c in range(N_CORES):
        dis_dst[c, :CORE_N] = dis[c * CORE_N:(c + 1) * CORE_N]
    dis_dst = dis_dst.astype(ml_dtypes.bfloat16)

    iota = np.tile(np.arange(128, dtype=np.float32).astype(ml_dtypes.bfloat16)[None, :],
                   (128, 1))

    per_core = []
    for c in range(N_CORES):
        per_core.append({
            "x": x_pad, "dis_pt": dis_pt, "dis_dst": dis_dst[c],
            "W_ext": W_ext, "iota": np.ascontiguousarray(iota),
            "eidx": eidx_w[c], "dstloc": dstl_pt[c],
        })
    return per_core, calls, T_bc, NT, E_plan


def _build_nc(calls, NT, E_plan, n_cores):
    import concourse.bacc as bacc
    import concourse.mybir as mybir
    import concourse.tile as tile
    from concourse import library_config

    bf16 = mybir.dt.bfloat16
    f32 = mybir.dt.float32

    nc = bacc.Bacc("TRN2", target_bir_lowering=False, debug=False,
                   num_devices=n_cores, num_swdge_queues=4)
    x_in = nc.dram_tensor("x", [NPAD, D], f32, kind="ExternalInput")
    dis_pt_in = nc.dram_tensor("dis_pt", [128, NXT], f32, kind="ExternalInput")
    dis_dst_in = nc.dram_tensor("dis_dst", [NBINS * BIN], bf16, kind="ExternalInput")
    W_in = nc.dram_tensor("W_ext", [65, 64], bf16, kind="ExternalInput")
    iota_in = nc.dram_tensor("iota", [128, 128], bf16, kind="ExternalInput")
    eidx_in = nc.dram_tensor("eidx", [128, E_plan // 16], mybir.dt.int16,
                             kind="ExternalInput")
    dstloc_in = nc.dram_tensor("dstloc", [128, NT], bf16, kind="ExternalInput")
    out_ext = nc.dram_tensor("out", [CORE_N, D], f32, kind="ExternalOutput")
    xs_ch = [nc.dram_tensor(f"xs{k}", [CHUNK_N, 128], bf16)
             for k in range(NCHUNKS)]   # internal scratch, per gather chunk

    x_v = x_in.ap().rearrange("(t p) f -> p t f", p=128)      # [128, NXT, 64]
    xs_v = [t.ap().rearrange("(t p) f -> p t f", p=128) for t in xs_ch]

    with tile.TileContext(nc) as tc, ExitStack() as ctx:
        nc.gpsimd.load_library(library_config.mlp)

        const_pool = ctx.enter_context(tc.tile_pool(name="consts", bufs=1))
        iota_sb = const_pool.tile([128, 128], bf16)
        nc.sync.dma_start(iota_sb[:], iota_in.ap())
        W_sb = const_pool.tile([64, 64], bf16)
        nc.sync.dma_start(W_sb[:], W_in.ap()[0:64, :])
        brow_sb = const_pool.tile([1, 64], bf16)
        nc.sync.dma_start(brow_sb[:], W_in.ap()[64:65, :])
        ones_sb = const_pool.tile([1, 128], bf16)
        nc.vector.memset(ones_sb[:], 1.0)
        dis_pt_sb = const_pool.tile([128, NXT], f32)
        nc.sync.dma_start(dis_pt_sb[:], dis_pt_in.ap())
        dis_rep = const_pool.tile([64, NBINS * BIN], bf16)
        nc.sync.dma_start(dis_rep[0:1, :], dis_dst_in.ap().unsqueeze(0))
        nc.gpsimd.partition_broadcast(dis_rep[:], dis_rep[0:1, :])
        eidx_sb = const_pool.tile([128, E_plan // 16], mybir.dt.int16)
        nc.sync.dma_start(eidx_sb[:], eidx_in.ap())
        dstloc_sb = const_pool.tile([128, NT], bf16)
        nc.sync.dma_start(dstloc_sb[:], dstloc_in.ap())

        # ---- phase 1: xs[n] = x[n] * dis[n], duplicated, bf16 ----
        # chunk-major so chunk-k gathers only wait on their own xs tensor;
        # pools stay open for the whole kernel so their SBUF addresses are
        # not recycled into the gather pools (address reuse would serialize
        # the first gather behind the entire xs phase).
        ch_t = CHUNK_N // 128   # node tiles per chunk
        xi_pool = ctx.enter_context(tc.tile_pool(name="xin", bufs=5))
        xo_pool = ctx.enter_context(tc.tile_pool(name="xout", bufs=3))
        for k in range(NCHUNKS):
            for i in range(ch_t // XS_STEP):
                gsl = slice(k * ch_t + i * XS_STEP,
                            k * ch_t + (i + 1) * XS_STEP)
                lsl = slice(i * XS_STEP, (i + 1) * XS_STEP)
                xin = xi_pool.tile([128, XS_STEP, 64], f32)
                nc.sync.dma_start(xin[:], x_v[:, gsl, :])
                xo = xo_pool.tile([128, XS_STEP, 64], bf16)
                dis_b = dis_pt_sb[:, gsl].unsqueeze(2).broadcast_to(
                    [128, XS_STEP, 64])
                nc.vector.tensor_tensor(xo[:], xin[:], dis_b,
                                        mybir.AluOpType.mult)
                nc.scalar.dma_start(xs_v[k][:, lsl, 0:64], xo[:])
                nc.scalar.dma_start(xs_v[k][:, lsl, 64:128], xo[:])

        # ---- phase 2: gather + one-hot matmul accumulate + epilogue ----
        max_call_tiles = max(sum(t for _, t in cells) for _, _, cells in calls)
        f_pool = ctx.enter_context(tc.tile_pool(name="fpool", bufs=6))
        o_pool = ctx.enter_context(tc.tile_pool(name="opool", bufs=4))
        agg_pool = ctx.enter_context(
            tc.tile_pool(name="aggpsum", bufs=GROUP, space="PSUM"))
        out_psum_pool = ctx.enter_context(
            tc.tile_pool(name="outpsum", bufs=2, space="PSUM"))
        epi_pool = ctx.enter_context(tc.tile_pool(name="epi", bufs=3))
        outsb_pool = ctx.enter_context(tc.tile_pool(name="outsb", bufs=3))

        qi = 0
        call_i = 0
        n_groups = (NBINS + GROUP - 1) // GROUP
        for gi in range(n_groups):
            bins = [b for b in range(gi * GROUP, min((gi + 1) * GROUP, NBINS))]
            psums = {bb: agg_pool.tile([64, 128], f32, tag="agg", name=f"agg_{bb}") for bb in bins}
            for k in range(NCHUNKS):
                ck, call_off, cells = calls[call_i]
                assert ck == k
                call_i += 1
                Tk = sum(t for _, t in cells)
                n_idx = Tk * 128
                F = f_pool.tile([128, Tk, 128], bf16, tag="F")
                nc.gpsimd.dma_gather(
                    F[:], xs_ch[k].ap(),
                    eidx_sb[:, call_off * 8:(call_off + Tk) * 8],
                    n_idx, n_idx, 128, single_packet=False, queue_num=qi % 4)
                qi += 1
                O = o_pool.tile([128, Tk, 128], bf16, tag="O")
                in0 = iota_sb[:].unsqueeze(1).broadcast_to([128, Tk, 128])
                in1 = dstloc_sb[:, call_off:call_off + Tk].unsqueeze(2) \
                    .broadcast_to([128, Tk, 128])
                nc.vector.tensor_tensor(O[:], in0, in1, mybir.AluOpType.is_equal)
                t = 0
                for bb, nt in cells:
                    for j in range(nt):
                        nc.tensor.matmul(
                            psums[bb][:], F[:, t, 0:64], O[:, t, :],
                            start=(k == 0 and j == 0),
                            stop=(k == NCHUNKS - 1 and j == nt - 1))
                        t += 1
            for bb in bins:
                aggTs = epi_pool.tile([64, 128], bf16, tag="aggTs")
                nc.vector.tensor_tensor(
                    aggTs[:], psums[bb][:],
                    dis_rep[:, bb * BIN:(bb + 1) * BIN],
                    mybir.AluOpType.mult)
                pout = out_psum_pool.tile([128, 64], f32, tag="pout")
                nc.tensor.matmul(pout[:], aggTs[:], W_sb[:],
                                 start=True, stop=False)
                nc.tensor.matmul(pout[:], ones_sb[:], brow_sb[:],
                                 start=False, stop=True)
                outt = outsb_pool.tile([128, 64], f32, tag="outt")
                nc.vector.tensor_relu(outt[:], pout[:])
                rows = min(BIN, CORE_N - bb * BIN)
                nc.scalar.dma_start(out_ext.ap()[bb * BIN:bb * BIN + rows, :],
                                  outt[0:rows, :])
    nc.compile()
    return nc


def _make_runner(nc, n_cores):
    import jax
    from jax.sharding import Mesh, PartitionSpec
    from jax.experimental.shard_map import shard_map
    import concourse.mybir as mybir
    from concourse.bass2jax import (_bass_exec_p, partition_id_tensor,
                                    install_neuronx_cc_hook)

    install_neuronx_cc_hook()
    partition_name = nc.partition_id_tensor.name if nc.partition_id_tensor else None
    in_names, out_names, out_avals, zero_outs = [], [], [], []
    for alloc in nc.m.functions[0].allocations:
        if not isinstance(alloc, mybir.MemoryLocationSet):
            continue
        name = alloc.memorylocations[0].name
        if alloc.kind == "ExternalInput":
            if name != partition_name:
                in_names.append(name)
        elif alloc.kind == "ExternalOutput":
            shape = tuple(alloc.tensor_shape)
            dtype = mybir.dt.np(alloc.dtype)
            out_names.append(name)
            out_avals.append(jax.core.ShapedArray(shape, dtype))
            zero_outs.append(np.zeros(shape, dtype))
    n_params = len(in_names)
    n_outs = len(out_avals)
    in_names_all = in_names + out_names
    if partition_name is not None:
        in_names_all = in_names_all + [partition_name]

    def _body(*args):
        operands = list(args)
        if partition_name is not None:
            operands.append(partition_id_tensor())
        outs = _bass_exec_p.bind(
            *operands, out_avals=tuple(out_avals), in_names=tuple(in_names_all),
            out_names=tuple(out_names), lowering_input_output_aliases=(),
            sim_require_finite=True, sim_require_nnan=True, nc=nc)
        return tuple(outs)

    devices = jax.devices()[:n_cores]
    mesh = Mesh(np.asarray(devices), ("core",))
    sharded = jax.jit(
        shard_map(_body, mesh=mesh,
                  in_specs=(PartitionSpec("core"),) * (n_params + n_outs),
                  out_specs=(PartitionSpec("core"),) * n_outs,
                  check_rep=False),
        keep_unused=True)

    def run(in_maps):
        per_core = [[np.asarray(m[name]) for name in in_names] for m in in_maps]
        concat_in = [np.concatenate([per_core[c][i] for c in range(n_cores)], axis=0)
                     for i in range(n_params)]
        concat_zeros = [np.zeros((n_cores * z.shape[0], *z.shape[1:]), z.dtype)
                        for z in zero_outs]
        args = [jax.device_put(a) for a in concat_in + concat_zeros]
        out = sharded(*args)
        jax.block_until_ready(out)
        results = [{name: np.asarray(out[i]).reshape(n_cores, *out_avals[i].shape)[c]
                    for i, name in enumerate(out_names)} for c in range(n_cores)]

        def rerun():
            o = sharded(*args)
            jax.block_until_ready(o)

        return results, rerun

    return run


_CACHE = {}


def kernel(x, edge_index, W, b):
    per_core, calls, T_bc, NT, E_plan = _host_prep(x, edge_index, W, b)
    key = (NT, E_plan)
    if key not in _CACHE:
        nc = _build_nc(calls, NT, E_plan, N_CORES)
        _CACHE[key] = (_make_runner(nc, N_CORES), nc)
    run, nc = _CACHE[key]
    results, rerun = run(per_core)
    kernel.last_rerun = rerun   # for profiling from test.py
    out = np.concatenate([results[c]["out"] for c in range(N_CORES)], axis=0)
    return np.ascontiguousarray(out[:N_NODES]).astype(np.float32)
